# revision 57
# baseline (speedup 1.0000x reference)
"""Trainium2 Bass kernel for nn_Attention_Module (sparse_attention).

Computation per batch b (x_b: [C=256, T=4096] fp32):
    energy = x_b @ x_b^T                      # (256, 256), K=4096
    attn   = softmax(rowmax(energy) - energy) # == exp(mu - e)/Z, mu = rowmin
    out    = gamma * (attn @ x_b) + x_b

Strategy (8 cores, pure data-parallel, 4 batches/core):
  - mm1 (energy) in fp16 from xt [P, KT, C] (x^T tiles, 8 MB/core).
  - mm2 (attn @ x) in fp8e4 with DoubleRow (2 fp8 MACs/PE cell): stationary
    At8 = P^T (exp matrix, values in (0,1]), moving xn8 = fp8(x) staged from
    host (4 MB/core instead of 8 MB fp16 -> 20 MB total HBM traffic).
  - gamma/Z folded into the PSUM evacuation scale; the +x residual is merged
    on host in fp32 during unshard (more precise than a device fp16 add).
  - Fine-grained software pipeline: each mm1 half-row k-loop hosts one mm2
    half (8 DoubleRow matmuls injected between k-tiles, order pinned with
    sync=False deps) so DVE/ACT drain PSUM while the PE stays dense. Only
    the last batch's second mm2 half runs bare at the end.
  - Batches 1-3 inputs are issued up-front on the idle GpSimd SWDGE queue;
    outputs go on the Sync HWDGE queue.
  - Cheap kernel exit: single sem-only butterfly instead of two full
    all-engine barriers.
"""

import numpy as np

B, C, T = 32, 256, 4096
NCORES = 8
NB = B // NCORES
P = 128
KT = T // P
KH = KT // 2
TC = T // 512
B0_BOUNDS = [0, 2, 4, 8, 12, 16, 24, 32]  # batch-0 graded chunk edges

_CACHE = {}


def _make_fast_tile_context(tile):
    """TileContext with a cheaper kernel exit: keep the final DMA drain but
    replace the two full all-engine barriers (per-engine InstDrain + double
    butterfly) with a single sem-only butterfly before the semaphore
    clear."""
    from concourse.vector_clock import ScopedClock

    class FastExitTileContext(tile.TileContext):
        def _drain_and_barrier(self, tick_clock, wait_clock):
            drain_inst = self.nc.sync.drain()
            wait_clock.add_sem_waits(
                drain_inst.ins, ScopedClock({None: tick_clock.global_clock})
            )
            self.nc.all_engine_barrier(sem_only=True)
            popped = self.nc._tile_sem_poison_stack.pop()
            assert popped is self._sem_poison
            self.nc.clear_and_free_semaphores(
                list(self.sems.allocated().values())
            )

    return FastExitTileContext


def _build_nc():
    from contextlib import ExitStack

    import concourse.bacc as bacc
    import concourse.bass as bass
    import concourse.tile as tile
    from concourse import mybir

    f32 = mybir.dt.float32
    f16 = mybir.dt.float16
    f8 = mybir.dt.float8e4
    DR = mybir.MatmulPerfMode.DoubleRow
    ts = bass.ts

    nc = bacc.Bacc(
        "TRN2",
        target_bir_lowering=False,
        debug=False,
        enable_asserts=False,
        num_devices=NCORES,
    )

    xt_h = nc.dram_tensor("xt", [NB, P, KT, C], f16, kind="ExternalInput")
    xn_h = nc.dram_tensor("xn", [NB, P, 2, T], f8, kind="ExternalInput")
    aux_h = nc.dram_tensor("aux", [P, 132], f32, kind="ExternalInput")
    o_h = nc.dram_tensor("o", [NB, C, T], f16, kind="ExternalOutput")

    FastExitTileContext = _make_fast_tile_context(tile)
    with FastExitTileContext(nc) as tc:
        with ExitStack() as ctx:
            singles = ctx.enter_context(tc.tile_pool(name="singles", bufs=1))
            xq_pool = ctx.enter_context(tc.tile_pool(name="xq", bufs=1))
            xt_pool = ctx.enter_context(tc.tile_pool(name="xt", bufs=3))
            xn_pool = ctx.enter_context(tc.tile_pool(name="xn", bufs=4))
            out_pool = ctx.enter_context(tc.tile_pool(name="out", bufs=4))
            att_pool = ctx.enter_context(tc.tile_pool(name="att", bufs=4))
            small = ctx.enter_context(tc.tile_pool(name="small", bufs=4))
            psum_e = ctx.enter_context(
                tc.tile_pool(name="psum_e", bufs=2, space="PSUM")
            )
            psum_t = ctx.enter_context(
                tc.tile_pool(name="psum_t", bufs=1, space="PSUM")
            )
            psum_o = ctx.enter_context(
                tc.tile_pool(name="psum_o", bufs=5, space="PSUM")
            )

            xt_ap = xt_h.ap()
            xn_ap = xn_h.ap()
            o_ap = o_h.ap()

            aux = singles.tile([P, 132], f32)
            nc.scalar.dma_start(aux[:], aux_h.ap())
            rgv = aux[:, 1:2]   # 1/gamma
            onev = aux[:, 2:3]  # 1.0
            ident = aux[:, 4:132]
            identf = singles.tile([P, P], f16)
            nc.vector.tensor_copy(identf[:], ident)

            def issue_loads(b):
                if b == 0:
                    # graded chunks on the Sync queue: first matmul starts
                    # after the first 128 KB lands
                    chunks = []
                    for ci in range(len(B0_BOUNDS) - 1):
                        lo, hi = B0_BOUNDS[ci], B0_BOUNDS[ci + 1]
                        t_ = xq_pool.tile(
                            [P, hi - lo, C], f16, tag=f"xq{ci}", name=f"xq{ci}"
                        )
                        nc.sync.dma_start(t_[:], xt_ap[b, :, lo:hi, :])
                        chunks.append((t_, lo, hi))
                    xn = xn_pool.tile([P, 2, T], f8, tag="xn", name="xn")
                    nc.sync.dma_start(xn[:], xn_ap[b])
                else:
                    xta = xt_pool.tile([P, KH, C], f16, tag="xta", name="xta")
                    xtb = xt_pool.tile([P, KH, C], f16, tag="xtb", name="xtb")
                    nc.sync.dma_start(xta[:], xt_ap[b, :, :KH, :])
                    nc.sync.dma_start(xtb[:], xt_ap[b, :, KH:, :])
                    chunks = [(xta, 0, KH), (xtb, KH, KT)]
                    xn = xn_pool.tile([P, 2, T], f8, tag="xn", name="xn")
                    nc.sync.dma_start(xn[:], xn_ap[b])
                return chunks, xn

            def mm2_step(job, t8, dma_every=4, out_q="sync"):
                """One DR matmul (K=256 packed) + evac of one 512-col output
                tile. Evacs alternate DVE/ACT, except the block's last tile
                goes to DVE so exp never queues behind an ACT evac."""
                pb, pAt, prZ, pxn, m, ot = job
                po = psum_o.tile([P, 512], f32)
                nc.tensor.matmul(
                    po[:],
                    lhsT=pAt[:, :, ts(m, P)],
                    rhs=pxn[:, :, ts(t8, 512)],
                    start=True,
                    stop=True,
                    perf_mode=DR,
                )
                if t8 % 2 == 0 or t8 == TC - 1:
                    nc.vector.tensor_scalar_mul(
                        ot[:, ts(t8, 512)], po[:], prZ[:, m : m + 1]
                    )
                else:
                    nc.scalar.mul(
                        ot[:, ts(t8, 512)], po[:], prZ[:, m : m + 1]
                    )
                if t8 % dma_every == dma_every - 1:
                    lo = (t8 - dma_every + 1) * 512
                    hi = (t8 + 1) * 512
                    eng = nc.scalar if out_q == "scalar" else nc.sync
                    eng.dma_start(
                        o_ap[pb].rearrange("(m p) t -> p m t", p=P)[
                            :, m, lo:hi
                        ],
                        ot[:, lo:hi],
                    )

            def make_job(pb, pAt, prZ, pxn, m):
                ot = out_pool.tile([P, T], f16, tag="ot", name="ot")
                return (pb, pAt, prZ, pxn, m, ot)

            tiles = {0: issue_loads(0)}
            state = {}

            for b in range(NB):
                xt, xn = tiles.pop(b)
                if b + 1 < NB:
                    tiles[b + 1] = issue_loads(b + 1)

                At = att_pool.tile([P, 2, C], f8)
                Zs = small.tile([P, 2], f32)
                Zb = small.tile([P, 2], f16)
                rZ = small.tile([P, 2], f32)
                jobs = state.pop(b - 1, []) if b > 0 else []

                if b == 0:
                    # batch 0's mm1 is input-bandwidth gated: fuse both
                    # half-row passes into one k-loop so each chunk is fully
                    # consumed on arrival and mm1 ends at last-chunk time
                    pes = [
                        psum_e.tile([P, C], f32, tag="pe", name=f"pe{fm}")
                        for fm in range(2)
                    ]
                    ci = 0
                    for k in range(KT):
                        while k >= xt[ci][2]:
                            ci += 1
                        src_t, lo, _ = xt[ci]
                        kk = k - lo
                        for fm in range(2):
                            nc.tensor.matmul(
                                pes[fm][:],
                                lhsT=src_t[:, kk, ts(fm, P)],
                                rhs=src_t[:, kk, :],
                                start=(k == 0),
                                stop=(k == KT - 1),
                                skip_group_check=True,
                            )

                for m in range(2):
                    # previous batch's mm2 half m rides inside this half's
                    # mm1 k-loop, front-loaded (one DR matmul every 2
                    # k-tiles) so the final 16 k-tiles give the evacuations
                    # runway to drain before the half boundary
                    hosted = jobs.pop(0) if jobs else None
                    if b == 0:
                        pe = pes[m]
                    else:
                        pe = psum_e.tile([P, C], f32)
                        ci = 0
                        for k in range(KT):
                            while k >= xt[ci][2]:
                                ci += 1
                            src_t, lo, _ = xt[ci]
                            kk = k - lo
                            nc.tensor.matmul(
                                pe[:],
                                lhsT=src_t[:, kk, ts(m, P)],
                                rhs=src_t[:, kk, :],
                                start=(k == 0),
                                stop=(k == KT - 1),
                                skip_group_check=True,
                            )
                            if hosted is not None and k % 2 == 1 and k < 16:
                                mm2_step(hosted, k // 2)
                            # last window also hosts the first half of the
                            # final batch's own m=0 mm2 in its late k-tiles
                            # (its exp/build chain is done ~2.7us earlier)
                            if (b == NB - 1 and m == 1 and k >= 25
                                    and k % 2 == 1):
                                mm2_step(state[b][0], (k - 25) // 2)
                    mu = small.tile([P, 1], f32)
                    nc.vector.tensor_reduce(
                        mu[:], pe[:], axis=mybir.AxisListType.X,
                        op=mybir.AluOpType.min,
                    )
                    Pm = small.tile([P, C], f16, tag=f"Pm{m}")
                    nc.scalar.activation(
                        Pm[:],
                        pe[:],
                        mybir.ActivationFunctionType.Exp,
                        bias=mu[:],
                        scale=-1.0,
                        accum_out=Zs[:, m : m + 1],
                    )
                    # Zb = Z/gamma (f16), rZ = gamma/Z (f32)
                    nc.vector.tensor_scalar_mul(
                        Zb[:, m : m + 1], Zs[:, m : m + 1], rgv
                    )
                    nc.vector.reciprocal(rZ[:, m : m + 1], Zb[:, m : m + 1])

                    # both transposes into one PSUM tile (PE writes to
                    # disjoint halves don't serialize), one combined evac
                    pt = psum_t.tile([P, 2 * P], f16)
                    for k in range(2):
                        nc.tensor.transpose(
                            pt[:, ts(k, P)], Pm[:, ts(k, P)], identf[:]
                        )
                    nc.scalar.mul(At[:, :, ts(m, P)], pt[:], onev)

                    # this half's mm2 runs during the next batch's mm1
                    state.setdefault(b, []).append(
                        make_job(b, At, rZ, xn, m)
                    )

            # tail: J(3,0)'s remaining tiles then J(3,1), with finer output
            # DMA alternating across both HWDGE queues for a faster drain
            j30, j31 = state.pop(NB - 1)
            for t8 in range(4, TC):
                mm2_step(j30, t8, dma_every=2,
                         out_q="scalar" if t8 == 5 else "sync")
            for t8 in range(TC):
                mm2_step(j31, t8, dma_every=2,
                         out_q="scalar" if t8 in (3, 7) else "sync")

    nc.compile()
    return nc


def _get_nc():
    if "nc" not in _CACHE:
        _CACHE["nc"] = _build_nc()
    return _CACHE["nc"]


def _make_aux(gamma_val):
    aux = np.zeros((P, 132), dtype=np.float32)
    aux[:, 0] = gamma_val
    aux[:, 1] = 1.0 / gamma_val if gamma_val != 0 else 0.0
    aux[:, 2] = 1.0
    aux[:, 4:132] = np.eye(P, dtype=np.float32)
    return aux


def kernel(x, gamma, _trace=False):
    import ml_dtypes

    import concourse.bass_utils as bass_utils

    x = np.ascontiguousarray(np.asarray(x, dtype=np.float32))
    gamma = np.asarray(gamma, dtype=np.float32).reshape(-1)

    nc = _get_nc()

    aux = _make_aux(gamma[0])
    x16 = x.astype(np.float16)
    in_maps = []
    for d in range(NCORES):
        xs16 = x16[d * NB : (d + 1) * NB]
        xt = np.ascontiguousarray(
            xs16.transpose(0, 2, 1).reshape(NB, KT, P, C).transpose(0, 2, 1, 3)
        )
        xs = x[d * NB : (d + 1) * NB]
        xn = np.ascontiguousarray(
            xs.reshape(NB, 2, P, T).transpose(0, 2, 1, 3)
        ).astype(ml_dtypes.float8_e4m3)
        in_maps.append({"xt": xt, "xn": xn, "aux": aux})

    res = bass_utils.run_bass_kernel_spmd(
        nc, in_maps, core_ids=list(range(NCORES)), trace=_trace
    )
    # device returns U = gamma * attn @ x (fp16); residual +x merged here
    out = np.concatenate([r["o"] for r in res.results], axis=0).astype(
        np.float32
    )
    out += x
    if _trace:
        _CACHE["last_results"] = res
    return out


# revision 62
# speedup vs baseline: 1.0359x; 1.0359x over previous
"""Trainium2 Bass kernel for nn_Attention_Module (sparse_attention).

Computation per batch b (x_b: [C=256, T=4096] fp32):
    energy = x_b @ x_b^T                      # (256, 256), K=4096
    attn   = softmax(rowmax(energy) - energy) # == exp(mu - e)/Z, mu = rowmin
    out    = gamma * (attn @ x_b) + x_b

Strategy (8 cores, pure data-parallel, 4 batches/core):
  - mm1 (energy) in fp16 from xt [P, KT, C] (x^T tiles, 8 MB/core).
  - mm2 (attn @ x) in fp8e4 with DoubleRow (2 fp8 MACs/PE cell): stationary
    At8 = P^T (exp matrix, values in (0,1]), moving xn8 = fp8(x) staged from
    host (4 MB/core instead of 8 MB fp16 -> 20 MB total HBM traffic).
  - gamma/Z folded into the PSUM evacuation scale; the +x residual is merged
    on host in fp32 during unshard (more precise than a device fp16 add).
  - Fine-grained software pipeline: each mm1 half-row k-loop hosts one mm2
    half (8 DoubleRow matmuls injected between k-tiles, order pinned with
    sync=False deps) so DVE/ACT drain PSUM while the PE stays dense. Only
    the last batch's second mm2 half runs bare at the end.
  - Batches 1-3 inputs are issued up-front on the idle GpSimd SWDGE queue;
    outputs go on the Sync HWDGE queue.
  - Cheap kernel exit: single sem-only butterfly instead of two full
    all-engine barriers.
"""

import numpy as np

B, C, T = 32, 256, 4096
NCORES = 8
NB = B // NCORES
P = 128
KT = T // P
KH = KT // 2
TC = T // 512
B0_BOUNDS = [0, 2, 4, 8, 12, 16, 24, 32]  # batch-0 graded chunk edges

_CACHE = {}


def _make_fast_tile_context(tile):
    """TileContext with a cheaper kernel exit: keep the final DMA drain but
    replace the two full all-engine barriers (per-engine InstDrain + double
    butterfly) with a single sem-only butterfly before the semaphore
    clear."""
    from concourse.vector_clock import ScopedClock

    class FastExitTileContext(tile.TileContext):
        def _drain_and_barrier(self, tick_clock, wait_clock):
            drain_inst = self.nc.sync.drain()
            wait_clock.add_sem_waits(
                drain_inst.ins, ScopedClock({None: tick_clock.global_clock})
            )
            self.nc.all_engine_barrier(sem_only=True)
            popped = self.nc._tile_sem_poison_stack.pop()
            assert popped is self._sem_poison
            self.nc.clear_and_free_semaphores(
                list(self.sems.allocated().values())
            )

    return FastExitTileContext


def _build_nc():
    from contextlib import ExitStack

    import concourse.bacc as bacc
    import concourse.bass as bass
    import concourse.tile as tile
    from concourse import mybir

    f32 = mybir.dt.float32
    f16 = mybir.dt.float16
    f8 = mybir.dt.float8e4
    DR = mybir.MatmulPerfMode.DoubleRow
    ts = bass.ts

    nc = bacc.Bacc(
        "TRN2",
        target_bir_lowering=False,
        debug=False,
        enable_asserts=False,
        num_devices=NCORES,
    )

    xt_h = nc.dram_tensor("xt", [NB, P, KT, C], f16, kind="ExternalInput")
    xn_h = nc.dram_tensor("xn", [NB, P, 2, T], f8, kind="ExternalInput")
    aux_h = nc.dram_tensor("aux", [P, 132], f32, kind="ExternalInput")
    o_h = nc.dram_tensor("o", [NB, C, T], f16, kind="ExternalOutput")

    FastExitTileContext = _make_fast_tile_context(tile)
    with FastExitTileContext(nc) as tc:
        with ExitStack() as ctx:
            singles = ctx.enter_context(tc.tile_pool(name="singles", bufs=1))
            xq_pool = ctx.enter_context(tc.tile_pool(name="xq", bufs=1))
            xt_pool = ctx.enter_context(tc.tile_pool(name="xt", bufs=3))
            xn_pool = ctx.enter_context(tc.tile_pool(name="xn", bufs=4))
            out_pool = ctx.enter_context(tc.tile_pool(name="out", bufs=6))
            att_pool = ctx.enter_context(tc.tile_pool(name="att", bufs=4))
            small = ctx.enter_context(tc.tile_pool(name="small", bufs=4))
            psum_e = ctx.enter_context(
                tc.tile_pool(name="psum_e", bufs=2, space="PSUM")
            )
            psum_t = ctx.enter_context(
                tc.tile_pool(name="psum_t", bufs=1, space="PSUM")
            )
            psum_o = ctx.enter_context(
                tc.tile_pool(name="psum_o", bufs=5, space="PSUM")
            )

            xt_ap = xt_h.ap()
            xn_ap = xn_h.ap()
            o_ap = o_h.ap()

            aux = singles.tile([P, 132], f32)
            nc.scalar.dma_start(aux[:], aux_h.ap())
            rgv = aux[:, 1:2]   # 1/gamma
            onev = aux[:, 2:3]  # 1.0
            ident = aux[:, 4:132]
            identf = singles.tile([P, P], f16)
            nc.vector.tensor_copy(identf[:], ident)

            def issue_loads(b):
                if b == 0:
                    # graded chunks on the Sync queue: first matmul starts
                    # after the first 128 KB lands
                    chunks = []
                    for ci in range(len(B0_BOUNDS) - 1):
                        lo, hi = B0_BOUNDS[ci], B0_BOUNDS[ci + 1]
                        t_ = xq_pool.tile(
                            [P, hi - lo, C], f16, tag=f"xq{ci}", name=f"xq{ci}"
                        )
                        nc.sync.dma_start(t_[:], xt_ap[b, :, lo:hi, :])
                        chunks.append((t_, lo, hi))
                    xn = xn_pool.tile([P, 2, T], f8, tag="xn", name="xn")
                    nc.sync.dma_start(xn[:], xn_ap[b])
                else:
                    xta = xt_pool.tile([P, KH, C], f16, tag="xta", name="xta")
                    xtb = xt_pool.tile([P, KH, C], f16, tag="xtb", name="xtb")
                    nc.sync.dma_start(xta[:], xt_ap[b, :, :KH, :])
                    nc.sync.dma_start(xtb[:], xt_ap[b, :, KH:, :])
                    chunks = [(xta, 0, KH), (xtb, KH, KT)]
                    xn = xn_pool.tile([P, 2, T], f8, tag="xn", name="xn")
                    nc.sync.dma_start(xn[:], xn_ap[b])
                return chunks, xn

            def mm2_step(job, t8, dma_every=4):
                """One DR matmul (K=256 packed) + evac of one 512-col output
                tile. Evacs alternate DVE/ACT, except the block's last tile
                goes to DVE so exp never queues behind an ACT evac."""
                pb, pAt, prZ, pxn, m, ot = job
                po = psum_o.tile([P, 512], f32)
                nc.tensor.matmul(
                    po[:],
                    lhsT=pAt[:, :, ts(m, P)],
                    rhs=pxn[:, :, ts(t8, 512)],
                    start=True,
                    stop=True,
                    perf_mode=DR,
                )
                if t8 % 2 == 0 or t8 == TC - 1:
                    nc.vector.tensor_scalar_mul(
                        ot[:, ts(t8, 512)], po[:], prZ[:, m : m + 1]
                    )
                else:
                    nc.scalar.mul(
                        ot[:, ts(t8, 512)], po[:], prZ[:, m : m + 1]
                    )
                if t8 % dma_every == dma_every - 1:
                    lo = (t8 - dma_every + 1) * 512
                    hi = (t8 + 1) * 512
                    nc.sync.dma_start(
                        o_ap[pb].rearrange("(m p) t -> p m t", p=P)[
                            :, m, lo:hi
                        ],
                        ot[:, lo:hi],
                    )

            def make_job(pb, pAt, prZ, pxn, m):
                ot = out_pool.tile([P, T], f16, tag="ot", name="ot")
                return (pb, pAt, prZ, pxn, m, ot)

            tiles = {0: issue_loads(0)}
            state = {}

            for b in range(NB):
                xt, xn = tiles.pop(b)
                if b + 1 < NB:
                    tiles[b + 1] = issue_loads(b + 1)

                At = att_pool.tile([P, 2, C], f8)
                Zs = small.tile([P, 2], f32)
                Zb = small.tile([P, 2], f16)
                rZ = small.tile([P, 2], f32)
                jobs = state.pop(b - 1, []) if b > 0 else []

                if b == 0:
                    # batch 0's mm1 is input-bandwidth gated: fuse both
                    # half-row passes into one k-loop so each chunk is fully
                    # consumed on arrival and mm1 ends at last-chunk time
                    pes = [
                        psum_e.tile([P, C], f32, tag="pe", name=f"pe{fm}")
                        for fm in range(2)
                    ]
                    ci = 0
                    for k in range(KT):
                        while k >= xt[ci][2]:
                            ci += 1
                        src_t, lo, _ = xt[ci]
                        kk = k - lo
                        for fm in range(2):
                            nc.tensor.matmul(
                                pes[fm][:],
                                lhsT=src_t[:, kk, ts(fm, P)],
                                rhs=src_t[:, kk, :],
                                start=(k == 0),
                                stop=(k == KT - 1),
                                skip_group_check=True,
                            )

                for m in range(2):
                    # previous batch's mm2 half m rides inside this half's
                    # mm1 k-loop, front-loaded (one DR matmul every 2
                    # k-tiles) so the final 16 k-tiles give the evacuations
                    # runway to drain before the half boundary
                    hosted = jobs.pop(0) if jobs else None
                    if b == 0:
                        pe = pes[m]
                    else:
                        pe = psum_e.tile([P, C], f32)
                        ci = 0
                        for k in range(KT):
                            while k >= xt[ci][2]:
                                ci += 1
                            src_t, lo, _ = xt[ci]
                            kk = k - lo
                            nc.tensor.matmul(
                                pe[:],
                                lhsT=src_t[:, kk, ts(m, P)],
                                rhs=src_t[:, kk, :],
                                start=(k == 0),
                                stop=(k == KT - 1),
                                skip_group_check=True,
                            )
                            if hosted is not None and k % 2 == 1 and k < 16:
                                mm2_step(hosted, k // 2)
                    mu = small.tile([P, 1], f32)
                    nc.vector.tensor_reduce(
                        mu[:], pe[:], axis=mybir.AxisListType.X,
                        op=mybir.AluOpType.min,
                    )
                    Pm = small.tile([P, C], f16, tag=f"Pm{m}")
                    nc.scalar.activation(
                        Pm[:],
                        pe[:],
                        mybir.ActivationFunctionType.Exp,
                        bias=mu[:],
                        scale=-1.0,
                        accum_out=Zs[:, m : m + 1],
                    )
                    # Zb = Z/gamma (f16), rZ = gamma/Z (f32)
                    nc.vector.tensor_scalar_mul(
                        Zb[:, m : m + 1], Zs[:, m : m + 1], rgv
                    )
                    nc.vector.reciprocal(rZ[:, m : m + 1], Zb[:, m : m + 1])

                    # both transposes into one PSUM tile (PE writes to
                    # disjoint halves don't serialize), one combined evac
                    pt = psum_t.tile([P, 2 * P], f16)
                    for k in range(2):
                        nc.tensor.transpose(
                            pt[:, ts(k, P)], Pm[:, ts(k, P)], identf[:]
                        )
                    nc.scalar.mul(At[:, :, ts(m, P)], pt[:], onev)

                    # this half's mm2 runs during the next batch's mm1
                    state.setdefault(b, []).append(
                        make_job(b, At, rZ, xn, m)
                    )

            # tail: the last batch's two halves run bare, with finer output
            # DMA so the store overlaps the remaining evacuations
            for job in state.pop(NB - 1):
                for t8 in range(TC):
                    mm2_step(job, t8, dma_every=2)

    nc.compile()
    return nc


def _get_nc():
    if "nc" not in _CACHE:
        _CACHE["nc"] = _build_nc()
    return _CACHE["nc"]


def _make_aux(gamma_val):
    aux = np.zeros((P, 132), dtype=np.float32)
    aux[:, 0] = gamma_val
    aux[:, 1] = 1.0 / gamma_val if gamma_val != 0 else 0.0
    aux[:, 2] = 1.0
    aux[:, 4:132] = np.eye(P, dtype=np.float32)
    return aux


def kernel(x, gamma, _trace=False):
    import ml_dtypes

    import concourse.bass_utils as bass_utils

    x = np.ascontiguousarray(np.asarray(x, dtype=np.float32))
    gamma = np.asarray(gamma, dtype=np.float32).reshape(-1)

    nc = _get_nc()

    aux = _make_aux(gamma[0])
    x16 = x.astype(np.float16)
    in_maps = []
    for d in range(NCORES):
        xs16 = x16[d * NB : (d + 1) * NB]
        xt = np.ascontiguousarray(
            xs16.transpose(0, 2, 1).reshape(NB, KT, P, C).transpose(0, 2, 1, 3)
        )
        xs = x[d * NB : (d + 1) * NB]
        xn = np.ascontiguousarray(
            xs.reshape(NB, 2, P, T).transpose(0, 2, 1, 3)
        ).astype(ml_dtypes.float8_e4m3)
        in_maps.append({"xt": xt, "xn": xn, "aux": aux})

    res = bass_utils.run_bass_kernel_spmd(
        nc, in_maps, core_ids=list(range(NCORES)), trace=_trace
    )
    # device returns U = gamma * attn @ x (fp16); residual +x merged here
    out = np.concatenate([r["o"] for r in res.results], axis=0).astype(
        np.float32
    )
    out += x
    if _trace:
        _CACHE["last_results"] = res
    return out
